# revision 30
# baseline (speedup 1.0000x reference)
"""Trainium2 Bass kernel for the non-local-attention block (nn_DNL_74234214744693).

Reference computation (B=4, C=64, H=W=64, N=H*W=4096):
    k = conv1x1(x,kw,kb); k_wh = k - mean_j(k)
    q = conv1x1(x,qw,qb)
    qk[b,i,j] = sum_c k_wh[b,c,i] q[b,c,j]      (q-mean drops: softmax-invariant)
    m  = conv1x1(x,mw,mb) -> [B,N];  mm[b,i,j] = m[b,i]*m[b,j]
    f  = softmax(qk, axis=-1) + softmax(mm, axis=0)   # second softmax over BATCH
    y  = einsum('bci,bij->bcj', v, f) + BN(conv1x1(x,ww,wb))

Structure of this implementation:
  * The host precomputes the four 1x1 convolutions (q, whitened k, v, m) in
    fp32 -- they are O(C^2 N) linear prep, ~3% of the FLOPs -- and ships them
    as bf16.  The device does all O(N^2) attention work: both exp fields,
    the batch-softmax normalization, and all the big matmuls.
  * Sharding: each of 8 cores owns a 512-row i-slice of the [N,N] maps for
    all 4 batch samples; partial y outputs are summed on the host.  The
    conv+BN residual is folded into the output matmul with weights
    pre-scaled by 1/8.
  * f1 = exp(qk)/2048 is stored fp8e4 (y1 = v1p@f1 is ~0.03% of |y|, so fp8
    is safe); the scale keeps exp below fp8e4's 240 max.  Row sums come for
    free from the activation accumulator, so v1p = v/rowsum needs no extra
    scaling.  f1 tiles are packed [128, 2, 4096] so the output matmul can use
    fp8 DoubleRow mode (two i-tiles contracted per instruction).
  * Engine budget (cost-model): ACT does only the mandatory 16.8M exps
    (~123us) and is the bottleneck; the batch-softmax chain (D-sum, recip,
    4 mults) is split DVE/Pool; PSUM->SBUF output staging is on Pool; output
    DMA goes straight from those staging tiles.
  * Schedule: e1 exps are it-major with jq-block 0's e2 units interleaved so
    the DVE/Pool pipeline has work the whole way through; phase B streams
    e2 units for jq1..3 behind the remaining matmuls with double-buffered
    PSUM output groups.
"""

import functools
import math

import numpy as np
import ml_dtypes

N_CORES = 8
B, C, H, W = 4, 64, 64, 64
N = H * W                 # 4096
SL = N // N_CORES         # 512  rows of the attention map per core
NIT = SL // 128           # 4    128-row i-tiles per core
NPP = NIT // 2            # 2    i-tile pairs (fp8 DoubleRow packing)
NJQ = 4                   # j-blocks in phase B
JQ = N // NJQ             # 1024
EPS = 1e-5

E1_BIAS = -math.log(2048.0)   # f1 = exp(qk)/2048 stays below fp8e4 max 240

BF16 = ml_dtypes.bfloat16

USE_DOUBLE_ROW = False


def _build_program():
    import concourse.bass as bass
    import concourse.tile as tile
    from concourse import bacc, mybir

    dt = mybir.dt
    AF = mybir.ActivationFunctionType
    ALU = mybir.AluOpType

    nc = bacc.Bacc("TRN2", target_bir_lowering=False, debug=False,
                   enable_asserts=False, num_devices=1)

    # ---------------- DRAM I/O ----------------
    # q is packed [128, N/2]: partitions 64h+c hold q[c, 2048h:2048(h+1)],
    # and kwh is duplicated on both partition halves so matmul bases align.
    q_ext = nc.dram_tensor("q_ext", [B, 128, N // 2], dt.bfloat16, kind="ExternalInput")
    kwh_ext = nc.dram_tensor("kwh_ext", [B, 128, SL], dt.bfloat16, kind="ExternalInput")
    vT_ext = nc.dram_tensor("vT_ext", [B, NIT, 128, C], dt.bfloat16, kind="ExternalInput")
    mcol_ext = nc.dram_tensor("mcol_ext", [B, 128, NIT], dt.float32, kind="ExternalInput")
    mb_ext = nc.dram_tensor("mb_ext", [B, N], dt.bfloat16, kind="ExternalInput")
    x_ext = nc.dram_tensor("x_ext", [B, C + 1, N], dt.bfloat16, kind="ExternalInput")
    wT_ext = nc.dram_tensor("wT_ext", [C + 1, C], dt.bfloat16, kind="ExternalInput")
    y_part = nc.dram_tensor("y_part", [B, C, N], dt.bfloat16, kind="ExternalOutput")

    with tile.TileContext(nc) as tc:
        from contextlib import ExitStack

        with ExitStack() as top:
            # ---------- persistent SBUF pools ----------
            consts = top.enter_context(tc.tile_pool(name="consts", bufs=1))
            p_q = top.enter_context(tc.tile_pool(name="p_q", bufs=B))
            p_kwh = top.enter_context(tc.tile_pool(name="p_kwh", bufs=B))
            p_vT = top.enter_context(tc.tile_pool(name="p_vT", bufs=B * NIT))
            p_v1p = top.enter_context(tc.tile_pool(name="p_v1p", bufs=B * NPP))
            p_mcol = top.enter_context(tc.tile_pool(name="p_mcol", bufs=B))
            p_f1 = top.enter_context(tc.tile_pool(name="p_f1", bufs=B * NPP))
            p_mbc = top.enter_context(tc.tile_pool(name="p_mbc", bufs=8))
            p_xw = top.enter_context(tc.tile_pool(name="p_xw", bufs=6))
            p_e2 = top.enter_context(tc.tile_pool(name="p_e2", bufs=24))
            p_t = top.enter_context(tc.tile_pool(name="p_t", bufs=4))
            p_d = top.enter_context(tc.tile_pool(name="p_d", bufs=2))
            p_r = top.enter_context(tc.tile_pool(name="p_r", bufs=2))
            p_rb = top.enter_context(tc.tile_pool(name="p_rb", bufs=2))
            p_zp = top.enter_context(tc.tile_pool(name="p_zp", bufs=12))
            p_out = top.enter_context(tc.tile_pool(name="p_out", bufs=4))

            sb_wT = consts.tile([C + 1, C], dt.bfloat16)
            nc.sync.dma_start(sb_wT, wT_ext.ap())
            e1_bias = consts.tile([128, 1], dt.float32, name="e1_bias")
            nc.vector.memset(e1_bias, E1_BIAS)

            q_sb = [p_q.tile([128, N // 2], dt.bfloat16, name=f"q{b}", tag="q") for b in range(B)]
            kwh = [p_kwh.tile([128, SL], dt.bfloat16, name=f"kwh{b}", tag="kwh") for b in range(B)]
            v_T = [[p_vT.tile([128, C], dt.bfloat16, name=f"vT{b}_{i}", tag="vT")
                    for i in range(NIT)] for b in range(B)]
            v1p = [[p_v1p.tile([128, 2, C], dt.float8e4, name=f"v1p{b}_{p}", tag="v1p")
                    for p in range(NPP)] for b in range(B)]
            mcol = [p_mcol.tile([128, NIT], dt.float32, name=f"mcol{b}", tag="mcol")
                    for b in range(B)]
            f1 = [[p_f1.tile([128, 2, N], dt.float8e4, name=f"f1_{b}_{p}", tag="f1")
                   for p in range(NPP)] for b in range(B)]

            # DMA order matters: the stream consumes q(b) at ~4us intervals,
            # so land kwh/q(b0)/mcol first, then the rest of q, then v.
            nc.sync.dma_start(kwh[0], kwh_ext.ap()[0])
            nc.sync.dma_start(q_sb[0], q_ext.ap()[0])
            for b in range(1, B):
                nc.sync.dma_start(kwh[b], kwh_ext.ap()[b])
            for b in range(B):
                nc.sync.dma_start(mcol[b], mcol_ext.ap()[b])

            m_bc = {}
            x_wx = {}

            def dma_jq(jq):
                for b in range(B):
                    t = p_mbc.tile([128, JQ], dt.bfloat16, name=f"mbc{b}_{jq}", tag="mbc")
                    jsl = slice(jq * JQ, (jq + 1) * JQ)
                    nc.sync.dma_start(t, mb_ext.ap()[b:b + 1, jsl].to_broadcast([128, JQ]))
                    m_bc[(b, jq)] = t
                for b in range(B):
                    t = p_xw.tile([C + 1, JQ], dt.bfloat16, name=f"xw{b}_{jq}", tag="xw")
                    nc.sync.dma_start(t, x_ext.ap()[b][:, jq * JQ:(jq + 1) * JQ])
                    x_wx[(b, jq)] = t

            dma_jq(0)
            for b in range(1, B):
                nc.sync.dma_start(q_sb[b], q_ext.ap()[b])
            for b in range(B):
                for it in range(NIT):
                    nc.sync.dma_start(v_T[b][it], vT_ext.ap()[b][it])
            dma_jq(1)

            # ---------- the batch-softmax unit for one (it, jq) ----------
            f2_tiles = {}

            def emit_e2_unit(it, jq):
                e2b = [p_e2.tile([128, JQ], dt.bfloat16, name=f"e2_{b}", tag="e2")
                       for b in range(B)]
                for b in range(B):
                    nc.scalar.activation(e2b[b], m_bc[(b, jq)], AF.Exp,
                                         scale=mcol[b][:, it:it + 1])
                t1 = p_t.tile([128, JQ], dt.bfloat16, name="t1", tag="t1")
                t2 = p_t.tile([128, JQ], dt.bfloat16, name="t2", tag="t2")
                nc.gpsimd.tensor_tensor(t1, e2b[0], e2b[1], op=ALU.add)
                nc.gpsimd.tensor_tensor(t2, t1, e2b[2], op=ALU.add)
                dD = p_d.tile([128, JQ], dt.float32, name="dD", tag="dD")
                rr = p_r.tile([128, JQ], dt.float32, name="rr", tag="rr")
                rrb = p_rb.tile([128, JQ], dt.bfloat16, name="rrb", tag="rrb")
                nc.vector.tensor_tensor(dD, t2, e2b[3], op=ALU.add)
                nc.vector.reciprocal_approx_fast(rr, dD)
                nc.vector.tensor_copy(rrb, rr)
                for b in range(B):
                    nc.vector.tensor_tensor(e2b[b], e2b[b], rrb, op=ALU.mult)
                f2_tiles[(it, jq)] = e2b

            # ---------- phase A: qk + e1 (it-major, full j), jq0 e2 units ----------
            # ACT runs e1 back-to-back (psP double-buffered); the jq0 e2
            # units interleave so DVE/Pool warm up; no output matmuls yet.
            # Phase A covers e1 only for j in [0, 2048) (the h=0 partition
            # half of the packed q); jq2/jq3's e1 streams inside phase B so
            # that region stays ACT-paced.  Z1 is estimated as 2x the half
            # row-sum (y1 is ~0.03% of |y|, so the sampling error is noise).
            with ExitStack() as phA:
                psP = phA.enter_context(tc.tile_pool(name="psP", bufs=2, space="PSUM"))
                for it in range(NIT):
                    for b in range(B):
                        ps = psP.tile([128, 2048], dt.float32, name="ps_qk", tag="psP")
                        for k4 in range(4):
                            nc.tensor.matmul(
                                ps[:, k4 * 512:(k4 + 1) * 512],
                                kwh[b][0:64, it * 128:(it + 1) * 128],
                                q_sb[b][0:64, k4 * 512:(k4 + 1) * 512],
                                start=True, stop=True)
                        zp = p_zp.tile([128, 1], dt.float32, name="zp", tag="zp")
                        nc.scalar.activation(
                            f1[b][it // 2][:, it % 2, 0:2048],
                            ps, AF.Exp, bias=e1_bias[:, :], accum_out=zp)
                        z1 = p_zp.tile([128, 1], dt.float32, name="z1", tag="z1")
                        rz = p_zp.tile([128, 1], dt.float32, name="rz", tag="rz")
                        nc.vector.tensor_scalar_mul(z1, zp, 2.0)
                        nc.vector.reciprocal_approx_fast(rz, z1)
                        nc.vector.tensor_scalar_mul(
                            v1p[b][it // 2][:, it % 2, :], v_T[b][it], rz)
                    emit_e2_unit(it, 0)
                    if it >= 2:
                        emit_e2_unit(it - 2, 1)

            # ---------- phase B: output matmuls + jq2/3 e1 + rest of e2 ----------
            with ExitStack() as phB:
                psPB = phB.enter_context(tc.tile_pool(name="psPB", bufs=2, space="PSUM"))
                psY = phB.enter_context(tc.tile_pool(name="psY", bufs=4, space="PSUM"))
                py_tiles = {}

                def pslice(jq, b, h):
                    t = py_tiles[jq][(b // 2) * 2 + h]
                    r0 = (b % 2) * 64
                    return t[r0:r0 + 64, :]

                def emit_wx(jq):
                    py_tiles[jq] = [
                        psY.tile([128, 512], dt.float32, name=f"py{pr}_{h}", tag="psY")
                        for pr in range(2) for h in range(2)]
                    for b in range(B):
                        for h in range(2):
                            cs = slice(h * 512, (h + 1) * 512)
                            nc.tensor.matmul(pslice(jq, b, h), sb_wT,
                                             x_wx[(b, jq)][:, cs],
                                             start=True, stop=False)

                def emit_f1_mms(jq, it):
                    if it == 0:
                        emit_wx(jq)
                    for b in range(B):
                        for h in range(2):
                            js = slice(jq * JQ + h * 512, jq * JQ + (h + 1) * 512)
                            if USE_DOUBLE_ROW and it % 2 == 1:
                                nc.tensor.matmul(
                                    pslice(jq, b, h), v1p[b][it // 2],
                                    f1[b][it // 2][:, :, js],
                                    start=False, stop=False,
                                    perf_mode=mybir.MatmulPerfMode.DoubleRow)
                            elif not USE_DOUBLE_ROW:
                                nc.tensor.matmul(
                                    pslice(jq, b, h), v1p[b][it // 2][:, it % 2, :],
                                    f1[b][it // 2][:, it % 2, js],
                                    start=False, stop=False)

                def emit_f2_mms(jq, it):
                    e2b = f2_tiles.pop((it, jq))
                    for b in range(B):
                        for h in range(2):
                            cs = slice(h * 512, (h + 1) * 512)
                            nc.tensor.matmul(pslice(jq, b, h), v_T[b][it],
                                             e2b[b][:, cs],
                                             start=False,
                                             stop=(it == NIT - 1))
                    if it == NIT - 1:
                        for pr in range(2):
                            for h in range(2):
                                out_sb = p_out.tile([128, 512], dt.bfloat16,
                                                    name="out_sb", tag="out")
                                if jq == NJQ - 1:
                                    nc.scalar.copy(out_sb, py_tiles[jq][pr * 2 + h])
                                else:
                                    nc.vector.tensor_copy(out_sb, py_tiles[jq][pr * 2 + h])
                                jsl = slice(jq * JQ + h * 512,
                                            jq * JQ + (h + 1) * 512)
                                nc.sync.dma_start(
                                    y_part.ap()[2 * pr:2 * pr + 2, :, jsl], out_sb)
                        del py_tiles[jq]

                # f1/f2 matmuls trail by one slot so PE always has ready work
                # queued ahead of any chain- or activation-gated block.
                f2_pending = []
                f1_pending = []
                for jq in range(NJQ):
                    if jq + 2 < NJQ:
                        dma_jq(jq + 2)
                    for it in range(NIT):
                        if jq >= 2:
                            # stream this jq's e1 chunks here (j-half 1)
                            qo = (jq % 2) * JQ
                            for b in range(B):
                                ps = psPB.tile([128, JQ], dt.float32,
                                               name="ps_qkB", tag="psPB")
                                for k2 in range(2):
                                    nc.tensor.matmul(
                                        ps[:, k2 * 512:(k2 + 1) * 512],
                                        kwh[b][64:128, it * 128:(it + 1) * 128],
                                        q_sb[b][64:128, qo + k2 * 512:qo + (k2 + 1) * 512],
                                        start=True, stop=True)
                                nc.scalar.activation(
                                    f1[b][it // 2][:, it % 2, jq * JQ:(jq + 1) * JQ],
                                    ps, AF.Exp, bias=e1_bias[:, :])
                        if jq >= 1 and (it, jq) not in f2_tiles:
                            emit_e2_unit(it, jq)
                        if f1_pending:
                            emit_f1_mms(*f1_pending.pop(0))
                        if f2_pending:
                            emit_f2_mms(*f2_pending.pop(0))
                        f1_pending.append((jq, it))
                        f2_pending.append((jq, it))
                for jq_it in f1_pending:
                    emit_f1_mms(*jq_it)
                for jq_it in f2_pending:
                    emit_f2_mms(*jq_it)

    nc.compile()
    return nc


@functools.lru_cache(maxsize=1)
def _get_program():
    return _build_program()


def _prep_inputs(inputs):
    x = np.asarray(inputs["x"], np.float32).reshape(B, C, N)
    qw = np.asarray(inputs["qw"], np.float32)
    qb = np.asarray(inputs["qb"], np.float32)
    kw = np.asarray(inputs["kw"], np.float32)
    mw = np.asarray(inputs["mw"], np.float32)
    mb = np.asarray(inputs["mb"], np.float32)
    vw = np.asarray(inputs["vw"], np.float32)
    vb = np.asarray(inputs["vb"], np.float32)
    ww = np.asarray(inputs["ww"], np.float32)
    wb = np.asarray(inputs["wb"], np.float32)
    g = np.asarray(inputs["bn_gamma"], np.float32)
    be = np.asarray(inputs["bn_beta"], np.float32)
    rm = np.asarray(inputs["bn_rm"], np.float32)
    rv = np.asarray(inputs["bn_rv"], np.float32)

    # host-side linear prep: the four 1x1 convs in fp32
    xu = x.mean(-1, keepdims=True)
    q = np.einsum("oc,bcn->bon", qw, x) + qb[None, :, None]
    k_wh = np.einsum("oc,bcn->bon", kw, x - xu)     # bias cancels in whitening
    v = np.einsum("oc,bcn->bon", vw, x) + vb[None, :, None]
    m = (np.einsum("oc,bcn->bon", mw, x) + mb[None, :, None])[:, 0]   # [B,N]

    ones = np.ones((B, 1, N), np.float32)
    x_ext = np.concatenate([x, ones], axis=1).astype(BF16)

    inv = g / np.sqrt(rv + EPS)
    wT = np.zeros((C + 1, C), np.float32)
    wT[:C, :] = (ww * inv[:, None]).T / N_CORES
    wT[C, :] = (wb * inv + be - rm * inv) / N_CORES

    # pack q [B,C,N] -> [B,128,N/2] (partition 64h+c = q[c, 2048h:2048(h+1)])
    q16 = np.ascontiguousarray(
        q.reshape(B, C, 2, N // 2).transpose(0, 2, 1, 3).reshape(B, 128, N // 2)
    ).astype(BF16)
    kwh16 = k_wh.astype(BF16)
    vT = np.ascontiguousarray(v.transpose(0, 2, 1)).astype(BF16)     # [B,N,C]
    m32 = m.astype(np.float32)

    common = {
        "q_ext": q16,
        "x_ext": x_ext,
        "mb_ext": m.astype(BF16),
        "wT_ext": wT.astype(BF16),
    }
    in_maps = []
    for ic in range(N_CORES):
        sl = slice(ic * SL, (ic + 1) * SL)
        mm = dict(common)
        kwh_sl = np.ascontiguousarray(kwh16[:, :, sl])
        mm["kwh_ext"] = np.concatenate([kwh_sl, kwh_sl], axis=1)
        mm["vT_ext"] = np.ascontiguousarray(vT[:, sl, :]).reshape(B, NIT, 128, C)
        mm["mcol_ext"] = np.ascontiguousarray(
            m32[:, sl].reshape(B, NIT, 128).transpose(0, 2, 1))
        in_maps.append(mm)
    return in_maps


def kernel(**inputs):
    from concourse.bass_utils import run_bass_kernel_spmd

    nc = _get_program()
    in_maps = _prep_inputs(inputs)
    res = run_bass_kernel_spmd(nc, in_maps, core_ids=list(range(N_CORES)))
    y = np.zeros((B, C, N), np.float32)
    for r in res.results:
        y += np.asarray(r["y_part"], dtype=np.float32)
    return y.reshape(B, C, H, W)


if __name__ == "__main__":
    rng = np.random.default_rng(0)
    ins = {
        "x": rng.standard_normal((B, C, H, W), dtype=np.float32),
        "qw": rng.standard_normal((C, C), dtype=np.float32) * 0.05,
        "qb": rng.standard_normal((C,), dtype=np.float32) * 0.05,
        "kw": rng.standard_normal((C, C), dtype=np.float32) * 0.05,
        "kb": rng.standard_normal((C,), dtype=np.float32) * 0.05,
        "mw": rng.standard_normal((1, C), dtype=np.float32) * 0.05,
        "mb": rng.standard_normal((1,), dtype=np.float32) * 0.05,
        "vw": rng.standard_normal((C, C), dtype=np.float32) * 0.05,
        "vb": rng.standard_normal((C,), dtype=np.float32) * 0.05,
        "ww": rng.standard_normal((C, C), dtype=np.float32) * 0.05,
        "wb": rng.standard_normal((C,), dtype=np.float32) * 0.05,
        "bn_gamma": np.ones((C,), np.float32),
        "bn_beta": np.zeros((C,), np.float32),
        "bn_rm": np.zeros((C,), np.float32),
        "bn_rv": np.ones((C,), np.float32),
    }
    out = kernel(**ins)
    print("kernel output", out.shape, out.dtype, np.abs(out).mean())


# revision 31
# speedup vs baseline: 1.0835x; 1.0835x over previous
"""Trainium2 Bass kernel for the non-local-attention block (nn_DNL_74234214744693).

Reference computation (B=4, C=64, H=W=64, N=H*W=4096):
    k = conv1x1(x,kw,kb); k_wh = k - mean_j(k)
    q = conv1x1(x,qw,qb)
    qk[b,i,j] = sum_c k_wh[b,c,i] q[b,c,j]      (q-mean drops: softmax-invariant)
    m  = conv1x1(x,mw,mb) -> [B,N];  mm[b,i,j] = m[b,i]*m[b,j]
    f  = softmax(qk, axis=-1) + softmax(mm, axis=0)   # second softmax over BATCH
    y  = einsum('bci,bij->bcj', v, f) + BN(conv1x1(x,ww,wb))

Structure of this implementation:
  * The host precomputes the four 1x1 convolutions (q, whitened k, v, m) in
    fp32 -- they are O(C^2 N) linear prep, ~3% of the FLOPs -- and ships them
    as bf16.  The device does all O(N^2) attention work: both exp fields,
    the batch-softmax normalization, and all the big matmuls.
  * Sharding: each of 8 cores owns a 512-row i-slice of the [N,N] maps for
    all 4 batch samples; partial y outputs are summed on the host.  The
    conv+BN residual is folded into the output matmul with weights
    pre-scaled by 1/8.
  * f1 = exp(qk)/2048 is stored fp8e4 (y1 = v1p@f1 is ~0.03% of |y|, so fp8
    is safe); the scale keeps exp below fp8e4's 240 max.  Row sums come for
    free from the activation accumulator, so v1p = v/rowsum needs no extra
    scaling.  f1 tiles are packed [128, 2, 4096] so the output matmul can use
    fp8 DoubleRow mode (two i-tiles contracted per instruction).
  * Engine budget (cost-model): ACT does only the mandatory 16.8M exps
    (~123us) and is the bottleneck; the batch-softmax chain (D-sum, recip,
    4 mults) is split DVE/Pool; PSUM->SBUF output staging is on Pool; output
    DMA goes straight from those staging tiles.
  * Schedule: e1 exps are it-major with jq-block 0's e2 units interleaved so
    the DVE/Pool pipeline has work the whole way through; phase B streams
    e2 units for jq1..3 behind the remaining matmuls with double-buffered
    PSUM output groups.
"""

import functools
import math

import numpy as np
import ml_dtypes

N_CORES = 8
B, C, H, W = 4, 64, 64, 64
N = H * W                 # 4096
SL = N // N_CORES         # 512  rows of the attention map per core
NIT = SL // 128           # 4    128-row i-tiles per core
NPP = NIT // 2            # 2    i-tile pairs (fp8 DoubleRow packing)
NJQ = 4                   # j-blocks in phase B
JQ = N // NJQ             # 1024
EPS = 1e-5

E1_BIAS = -math.log(2048.0)   # f1 = exp(qk)/2048 stays below fp8e4 max 240

BF16 = ml_dtypes.bfloat16

USE_DOUBLE_ROW = False


def _build_program():
    import concourse.bass as bass
    import concourse.tile as tile
    from concourse import bacc, mybir

    dt = mybir.dt
    AF = mybir.ActivationFunctionType
    ALU = mybir.AluOpType

    nc = bacc.Bacc("TRN2", target_bir_lowering=False, debug=False,
                   enable_asserts=False, num_devices=1)

    # ---------------- DRAM I/O ----------------
    # q is packed [128, N/2]: partitions 64h+c hold q[c, 2048h:2048(h+1)],
    # and kwh is duplicated on both partition halves so matmul bases align.
    q_ext = nc.dram_tensor("q_ext", [B, 128, N // 2], dt.bfloat16, kind="ExternalInput")
    kwh_ext = nc.dram_tensor("kwh_ext", [B, 128, SL], dt.bfloat16, kind="ExternalInput")
    vT_ext = nc.dram_tensor("vT_ext", [B, NIT, 128, C], dt.bfloat16, kind="ExternalInput")
    mcol_ext = nc.dram_tensor("mcol_ext", [B, 128, NIT], dt.float32, kind="ExternalInput")
    mb_ext = nc.dram_tensor("mb_ext", [B, N], dt.bfloat16, kind="ExternalInput")
    x_ext = nc.dram_tensor("x_ext", [B, C + 1, N], dt.bfloat16, kind="ExternalInput")
    wT_ext = nc.dram_tensor("wT_ext", [C + 1, C], dt.bfloat16, kind="ExternalInput")
    y_part = nc.dram_tensor("y_part", [B, C, N], dt.bfloat16, kind="ExternalOutput")

    with tile.TileContext(nc) as tc:
        from contextlib import ExitStack

        with ExitStack() as top:
            # ---------- persistent SBUF pools ----------
            consts = top.enter_context(tc.tile_pool(name="consts", bufs=1))
            p_q = top.enter_context(tc.tile_pool(name="p_q", bufs=B))
            p_kwh = top.enter_context(tc.tile_pool(name="p_kwh", bufs=B))
            p_vT = top.enter_context(tc.tile_pool(name="p_vT", bufs=B * NIT))
            p_v1p = top.enter_context(tc.tile_pool(name="p_v1p", bufs=B * NPP))
            p_mcol = top.enter_context(tc.tile_pool(name="p_mcol", bufs=B))
            p_f1 = top.enter_context(tc.tile_pool(name="p_f1", bufs=B * NPP))
            p_mbc = top.enter_context(tc.tile_pool(name="p_mbc", bufs=8))
            p_xw = top.enter_context(tc.tile_pool(name="p_xw", bufs=6))
            p_e2 = top.enter_context(tc.tile_pool(name="p_e2", bufs=24))
            p_t = top.enter_context(tc.tile_pool(name="p_t", bufs=4))
            p_d = top.enter_context(tc.tile_pool(name="p_d", bufs=2))
            p_r = top.enter_context(tc.tile_pool(name="p_r", bufs=2))
            p_rb = top.enter_context(tc.tile_pool(name="p_rb", bufs=2))
            p_zp = top.enter_context(tc.tile_pool(name="p_zp", bufs=12))
            p_out = top.enter_context(tc.tile_pool(name="p_out", bufs=4))

            sb_wT = consts.tile([C + 1, C], dt.bfloat16)
            nc.sync.dma_start(sb_wT, wT_ext.ap())
            e1_bias = consts.tile([128, 1], dt.float32, name="e1_bias")
            nc.vector.memset(e1_bias, E1_BIAS)

            q_sb = [p_q.tile([128, N // 2], dt.bfloat16, name=f"q{b}", tag="q") for b in range(B)]
            kwh = [p_kwh.tile([128, SL], dt.bfloat16, name=f"kwh{b}", tag="kwh") for b in range(B)]
            v_T = [[p_vT.tile([128, C], dt.bfloat16, name=f"vT{b}_{i}", tag="vT")
                    for i in range(NIT)] for b in range(B)]
            v1p = [[p_v1p.tile([128, 2, C], dt.float8e4, name=f"v1p{b}_{p}", tag="v1p")
                    for p in range(NPP)] for b in range(B)]
            mcol = [p_mcol.tile([128, NIT], dt.float32, name=f"mcol{b}", tag="mcol")
                    for b in range(B)]
            f1 = [[p_f1.tile([128, 2, N], dt.float8e4, name=f"f1_{b}_{p}", tag="f1")
                   for p in range(NPP)] for b in range(B)]

            # DMA order matters: the stream consumes q(b) at ~4us intervals,
            # so land kwh/q(b0)/mcol first, then the rest of q, then v.
            nc.sync.dma_start(kwh[0], kwh_ext.ap()[0])
            nc.sync.dma_start(q_sb[0], q_ext.ap()[0])
            for b in range(1, B):
                nc.sync.dma_start(kwh[b], kwh_ext.ap()[b])
            for b in range(B):
                nc.sync.dma_start(mcol[b], mcol_ext.ap()[b])

            m_bc = {}
            x_wx = {}

            def dma_jq(jq):
                for b in range(B):
                    t = p_mbc.tile([128, JQ], dt.bfloat16, name=f"mbc{b}_{jq}", tag="mbc")
                    jsl = slice(jq * JQ, (jq + 1) * JQ)
                    nc.sync.dma_start(t, mb_ext.ap()[b:b + 1, jsl].to_broadcast([128, JQ]))
                    m_bc[(b, jq)] = t
                for b in range(B):
                    t = p_xw.tile([C + 1, JQ], dt.bfloat16, name=f"xw{b}_{jq}", tag="xw")
                    nc.sync.dma_start(t, x_ext.ap()[b][:, jq * JQ:(jq + 1) * JQ])
                    x_wx[(b, jq)] = t

            dma_jq(0)
            for b in range(1, B):
                nc.sync.dma_start(q_sb[b], q_ext.ap()[b])
            for b in range(B):
                for it in range(NIT):
                    nc.sync.dma_start(v_T[b][it], vT_ext.ap()[b][it])
            dma_jq(1)

            # ---------- the batch-softmax unit for one (it, jq) ----------
            f2_tiles = {}

            def emit_e2_unit(it, jq):
                e2b = [p_e2.tile([128, JQ], dt.bfloat16, name=f"e2_{b}", tag="e2")
                       for b in range(B)]
                for b in range(B):
                    nc.scalar.activation(e2b[b], m_bc[(b, jq)], AF.Exp,
                                         scale=mcol[b][:, it:it + 1])
                t1 = p_t.tile([128, JQ], dt.bfloat16, name="t1", tag="t1")
                t2 = p_t.tile([128, JQ], dt.bfloat16, name="t2", tag="t2")
                nc.gpsimd.tensor_tensor(t1, e2b[0], e2b[1], op=ALU.add)
                nc.gpsimd.tensor_tensor(t2, t1, e2b[2], op=ALU.add)
                dD = p_d.tile([128, JQ], dt.float32, name="dD", tag="dD")
                rr = p_r.tile([128, JQ], dt.float32, name="rr", tag="rr")
                rrb = p_rb.tile([128, JQ], dt.bfloat16, name="rrb", tag="rrb")
                nc.vector.tensor_tensor(dD, t2, e2b[3], op=ALU.add)
                nc.vector.reciprocal_approx_fast(rr, dD)
                nc.vector.tensor_copy(rrb, rr)
                for b in range(B):
                    nc.vector.tensor_tensor(e2b[b], e2b[b], rrb, op=ALU.mult)
                f2_tiles[(it, jq)] = e2b

            # ---------- phase A: qk + e1 (it-major, full j), jq0 e2 units ----------
            # ACT runs e1 back-to-back (psP double-buffered); the jq0 e2
            # units interleave so DVE/Pool warm up; no output matmuls yet.
            with ExitStack() as phA:
                psP = phA.enter_context(tc.tile_pool(name="psP", bufs=2, space="PSUM"))
                for it in range(NIT):
                    for b in range(B):
                        zp = [p_zp.tile([128, 1], dt.float32, name=f"zp{h}", tag="zp")
                              for h in range(2)]
                        for h in range(2):
                            ps = psP.tile([128, 2048], dt.float32, name="ps_qk", tag="psP")
                            pb = slice(64 * h, 64 * h + 64)
                            for k4 in range(4):
                                nc.tensor.matmul(
                                    ps[:, k4 * 512:(k4 + 1) * 512],
                                    kwh[b][pb, it * 128:(it + 1) * 128],
                                    q_sb[b][pb, k4 * 512:(k4 + 1) * 512],
                                    start=True, stop=True)
                            nc.scalar.activation(
                                f1[b][it // 2][:, it % 2, h * 2048:(h + 1) * 2048],
                                ps, AF.Exp, bias=e1_bias[:, :], accum_out=zp[h])
                        z1 = p_zp.tile([128, 1], dt.float32, name="z1", tag="z1")
                        rz = p_zp.tile([128, 1], dt.float32, name="rz", tag="rz")
                        nc.vector.tensor_tensor(z1, zp[0], zp[1], op=ALU.add)
                        nc.vector.reciprocal_approx_fast(rz, z1)
                        nc.vector.tensor_scalar_mul(
                            v1p[b][it // 2][:, it % 2, :], v_T[b][it], rz)
                    emit_e2_unit(it, 0)
                    if it >= 2:
                        emit_e2_unit(it - 2, 1)

            # ---------- phase B: output matmuls + remaining e2 units ----------
            with ExitStack() as phB:
                psY = phB.enter_context(tc.tile_pool(name="psY", bufs=8, space="PSUM"))
                py_tiles = {}

                def pslice(jq, b, h):
                    t = py_tiles[jq][(b // 2) * 2 + h]
                    r0 = (b % 2) * 64
                    return t[r0:r0 + 64, :]

                def emit_wx(jq):
                    py_tiles[jq] = [
                        psY.tile([128, 512], dt.float32, name=f"py{pr}_{h}", tag="psY")
                        for pr in range(2) for h in range(2)]
                    for b in range(B):
                        for h in range(2):
                            cs = slice(h * 512, (h + 1) * 512)
                            nc.tensor.matmul(pslice(jq, b, h), sb_wT,
                                             x_wx[(b, jq)][:, cs],
                                             start=True, stop=False)

                def emit_f1_mms(jq, it):
                    if it == 0:
                        emit_wx(jq)
                    for b in range(B):
                        for h in range(2):
                            js = slice(jq * JQ + h * 512, jq * JQ + (h + 1) * 512)
                            if USE_DOUBLE_ROW and it % 2 == 1:
                                nc.tensor.matmul(
                                    pslice(jq, b, h), v1p[b][it // 2],
                                    f1[b][it // 2][:, :, js],
                                    start=False, stop=False,
                                    perf_mode=mybir.MatmulPerfMode.DoubleRow)
                            elif not USE_DOUBLE_ROW:
                                nc.tensor.matmul(
                                    pslice(jq, b, h), v1p[b][it // 2][:, it % 2, :],
                                    f1[b][it // 2][:, it % 2, js],
                                    start=False, stop=False)

                def emit_f2_mms(jq, it):
                    e2b = f2_tiles.pop((it, jq))
                    for b in range(B):
                        for h in range(2):
                            cs = slice(h * 512, (h + 1) * 512)
                            nc.tensor.matmul(pslice(jq, b, h), v_T[b][it],
                                             e2b[b][:, cs],
                                             start=False,
                                             stop=(it == NIT - 1))
                    if it == NIT - 1:
                        for pr in range(2):
                            for h in range(2):
                                out_sb = p_out.tile([128, 512], dt.bfloat16,
                                                    name="out_sb", tag="out")
                                if jq == NJQ - 1:
                                    nc.scalar.copy(out_sb, py_tiles[jq][pr * 2 + h])
                                else:
                                    nc.vector.tensor_copy(out_sb, py_tiles[jq][pr * 2 + h])
                                jsl = slice(jq * JQ + h * 512,
                                            jq * JQ + (h + 1) * 512)
                                nc.sync.dma_start(
                                    y_part.ap()[2 * pr:2 * pr + 2, :, jsl], out_sb)
                        del py_tiles[jq]

                # f2 matmuls trail their unit by one slot so PE always has
                # ready f1 work queued ahead of a chain-gated f2 block.
                f2_pending = []
                for jq in range(NJQ):
                    if jq + 2 < NJQ:
                        dma_jq(jq + 2)
                    for it in range(NIT):
                        emit_f1_mms(jq, it)
                        if jq >= 1 and (it, jq) not in f2_tiles:
                            emit_e2_unit(it, jq)
                        if f2_pending:
                            emit_f2_mms(*f2_pending.pop(0))
                        f2_pending.append((jq, it))
                for jq_it in f2_pending:
                    emit_f2_mms(*jq_it)

    nc.compile()
    return nc


@functools.lru_cache(maxsize=1)
def _get_program():
    return _build_program()


def _prep_inputs(inputs):
    x = np.asarray(inputs["x"], np.float32).reshape(B, C, N)
    qw = np.asarray(inputs["qw"], np.float32)
    qb = np.asarray(inputs["qb"], np.float32)
    kw = np.asarray(inputs["kw"], np.float32)
    mw = np.asarray(inputs["mw"], np.float32)
    mb = np.asarray(inputs["mb"], np.float32)
    vw = np.asarray(inputs["vw"], np.float32)
    vb = np.asarray(inputs["vb"], np.float32)
    ww = np.asarray(inputs["ww"], np.float32)
    wb = np.asarray(inputs["wb"], np.float32)
    g = np.asarray(inputs["bn_gamma"], np.float32)
    be = np.asarray(inputs["bn_beta"], np.float32)
    rm = np.asarray(inputs["bn_rm"], np.float32)
    rv = np.asarray(inputs["bn_rv"], np.float32)

    # host-side linear prep: the four 1x1 convs in fp32
    xu = x.mean(-1, keepdims=True)
    q = np.einsum("oc,bcn->bon", qw, x) + qb[None, :, None]
    k_wh = np.einsum("oc,bcn->bon", kw, x - xu)     # bias cancels in whitening
    v = np.einsum("oc,bcn->bon", vw, x) + vb[None, :, None]
    m = (np.einsum("oc,bcn->bon", mw, x) + mb[None, :, None])[:, 0]   # [B,N]

    ones = np.ones((B, 1, N), np.float32)
    x_ext = np.concatenate([x, ones], axis=1).astype(BF16)

    inv = g / np.sqrt(rv + EPS)
    wT = np.zeros((C + 1, C), np.float32)
    wT[:C, :] = (ww * inv[:, None]).T / N_CORES
    wT[C, :] = (wb * inv + be - rm * inv) / N_CORES

    # pack q [B,C,N] -> [B,128,N/2] (partition 64h+c = q[c, 2048h:2048(h+1)])
    q16 = np.ascontiguousarray(
        q.reshape(B, C, 2, N // 2).transpose(0, 2, 1, 3).reshape(B, 128, N // 2)
    ).astype(BF16)
    kwh16 = k_wh.astype(BF16)
    vT = np.ascontiguousarray(v.transpose(0, 2, 1)).astype(BF16)     # [B,N,C]
    m32 = m.astype(np.float32)

    common = {
        "q_ext": q16,
        "x_ext": x_ext,
        "mb_ext": m.astype(BF16),
        "wT_ext": wT.astype(BF16),
    }
    in_maps = []
    for ic in range(N_CORES):
        sl = slice(ic * SL, (ic + 1) * SL)
        mm = dict(common)
        kwh_sl = np.ascontiguousarray(kwh16[:, :, sl])
        mm["kwh_ext"] = np.concatenate([kwh_sl, kwh_sl], axis=1)
        mm["vT_ext"] = np.ascontiguousarray(vT[:, sl, :]).reshape(B, NIT, 128, C)
        mm["mcol_ext"] = np.ascontiguousarray(
            m32[:, sl].reshape(B, NIT, 128).transpose(0, 2, 1))
        in_maps.append(mm)
    return in_maps


def kernel(**inputs):
    from concourse.bass_utils import run_bass_kernel_spmd

    nc = _get_program()
    in_maps = _prep_inputs(inputs)
    res = run_bass_kernel_spmd(nc, in_maps, core_ids=list(range(N_CORES)))
    y = np.zeros((B, C, N), np.float32)
    for r in res.results:
        y += np.asarray(r["y_part"], dtype=np.float32)
    return y.reshape(B, C, H, W)


if __name__ == "__main__":
    rng = np.random.default_rng(0)
    ins = {
        "x": rng.standard_normal((B, C, H, W), dtype=np.float32),
        "qw": rng.standard_normal((C, C), dtype=np.float32) * 0.05,
        "qb": rng.standard_normal((C,), dtype=np.float32) * 0.05,
        "kw": rng.standard_normal((C, C), dtype=np.float32) * 0.05,
        "kb": rng.standard_normal((C,), dtype=np.float32) * 0.05,
        "mw": rng.standard_normal((1, C), dtype=np.float32) * 0.05,
        "mb": rng.standard_normal((1,), dtype=np.float32) * 0.05,
        "vw": rng.standard_normal((C, C), dtype=np.float32) * 0.05,
        "vb": rng.standard_normal((C,), dtype=np.float32) * 0.05,
        "ww": rng.standard_normal((C, C), dtype=np.float32) * 0.05,
        "wb": rng.standard_normal((C,), dtype=np.float32) * 0.05,
        "bn_gamma": np.ones((C,), np.float32),
        "bn_beta": np.zeros((C,), np.float32),
        "bn_rm": np.zeros((C,), np.float32),
        "bn_rv": np.ones((C,), np.float32),
    }
    out = kernel(**ins)
    print("kernel output", out.shape, out.dtype, np.abs(out).mean())
